# revision 1
# baseline (speedup 1.0000x reference)
"""Adaptive-softmax logits kernel for trn2 (8 NeuronCores, SPMD).

Problem: out = concat([hidden @ head_w,
                       ((hidden @ down0) @ dec0) * m0,
                       ((hidden @ down1) @ dec1) * m1], axis=1)
with hidden [2048, 1024], head_w [1024, 2002], dec0 [1024, 8000],
dec1 [256, 40000]; m0/m1 are per-row cluster masks from `target`.

Sharding: vocab-parallel. Each core gets 1/8 of every output segment
(head padded 2002->2048 so each core takes 256 head + 1000 t0 + 5000 t1
columns) and computes the down-projections (h0 = hidden @ down0,
h1 = hidden @ down1) redundantly. This balances both HBM traffic
(output bytes dominate; equal columns per core) and PE time (equal
K-weighted columns per core).

On-device layout: everything is computed as out[b, v] = lhsT.T @ rhs
with lhsT = x^T k-chunk [128, 128] stationary and rhs = W [128, ~500]
moving, so output tiles land batch-major and DMA out contiguously.
hidden is fed pre-transposed (hT) from the host. All matmul operands
use dtype float32r: fp32 storage, 1 cycle/row on the PE at free-dim
>= 256 (4x faster than fp32) with ~1.5e-4 absmax relative error.

Masks are folded into the PSUM->SBUF eviction of h0^T / h1^T
(tensor_mul with a host-broadcast [128, B] mask), so decode outputs
need no separate masking pass.
"""

import numpy as np

import concourse.mybir as mybir
import concourse.tile as tile
from concourse import bacc
from concourse.bass_utils import run_bass_kernel_spmd

# Problem shapes (hardcoded per the grading contract).
B = 2048  # batch
H = 1024  # hidden
NCORES = 8
P = 128
KC = H // P  # 8 k-chunks for K=1024 contractions
HEAD = 2002
HEAD_PAD = 2048  # padded so each core gets 256 head columns
T0 = 8000  # cluster-0 decode width (dec0 columns)
T1 = 40000  # cluster-1 decode width (dec1 columns)
R1 = 256  # tail-1 down-projection width (down1 columns)
KC1 = R1 // P  # 2 k-chunks for the t1 decode contraction

# Per-core column counts.
HEAD_C = HEAD_PAD // NCORES  # 256
T0_C = T0 // NCORES  # 1000
T1_C = T1 // NCORES  # 5000
OUT_C = HEAD_C + T0_C + T1_C  # 6256

# Batch chunking: stream hT / h0T / h1T in chunks of 256 rows.
BC = 256  # batch chunk
NCHUNK = B // BC  # 8
BT = BC // P  # 2 psum batch-tiles per chunk

VT = 500  # decode free-dim tile (>=256 keeps f32r at 1 cyc/row)
T0_VT = T0_C // VT  # 2
T1_VT = T1_C // VT  # 10

F32 = mybir.dt.float32
F32R = mybir.dt.float32r

_compiled = None  # (nc, names) cache so repeat kernel() calls skip rebuild


def _build():
    nc = bacc.Bacc(None)

    hT = nc.declare_dram_parameter("hT", [H, B], F32R, isOutput=False)
    wh = nc.declare_dram_parameter("wh", [H, HEAD_C], F32R, isOutput=False)
    down0 = nc.declare_dram_parameter("down0", [H, H], F32R, isOutput=False)
    down1 = nc.declare_dram_parameter("down1", [H, R1], F32R, isOutput=False)
    d0 = nc.declare_dram_parameter("d0", [H, T0_C], F32R, isOutput=False)
    d1 = nc.declare_dram_parameter("d1", [R1, T1_C], F32R, isOutput=False)
    m0b = nc.declare_dram_parameter("m0b", [P, B], F32, isOutput=False)
    m1b = nc.declare_dram_parameter("m1b", [P, B], F32, isOutput=False)
    out = nc.declare_dram_parameter("out", [B, OUT_C], F32, isOutput=True)

    hT3 = hT.rearrange("(ko p) b -> p ko b", p=P)

    with tile.TileContext(nc) as tc:
        with (
            tc.tile_pool(name="consts", bufs=1) as consts,
            tc.tile_pool(name="stream", bufs=2) as stream,
            tc.tile_pool(name="hpool", bufs=1) as hpool,
            tc.tile_pool(name="opool", bufs=4) as opool,
            tc.tile_pool(name="psum", bufs=4, space="PSUM") as psum,
        ):
            # Resident weights.
            down0_sb = consts.tile([P, KC, H], F32R)
            nc.sync.dma_start(down0_sb[:], down0.rearrange("(ko p) m -> p ko m", p=P))
            down1_sb = consts.tile([P, KC, R1], F32R)
            nc.sync.dma_start(down1_sb[:], down1.rearrange("(ko p) m -> p ko m", p=P))
            wh_sb = consts.tile([P, KC, HEAD_C], F32R)
            nc.sync.dma_start(wh_sb[:], wh.rearrange("(ko p) v -> p ko v", p=P))
            d0_sb = consts.tile([P, KC, T0_C], F32R)
            nc.sync.dma_start(d0_sb[:], d0.rearrange("(ko p) v -> p ko v", p=P))
            d1_sb = consts.tile([P, KC1, T1_C], F32R)
            nc.sync.dma_start(d1_sb[:], d1.rearrange("(ko p) v -> p ko v", p=P))

            for bc in range(NCHUNK):
                bsl = slice(bc * BC, (bc + 1) * BC)

                hTc = stream.tile([P, KC, BC], F32R, tag="hTc")
                nc.sync.dma_start(hTc[:], hT3[:, :, bsl])
                m0c = stream.tile([P, BC], F32, tag="m0c", bufs=1)
                nc.sync.dma_start(m0c[:], m0b[:, bsl])
                m1c = stream.tile([P, BC], F32, tag="m1c", bufs=1)
                nc.sync.dma_start(m1c[:], m1b[:, bsl])

                # Down-projections: h0T[m, b] = sum_k down0[k, m] hT[k, b],
                # masked by m0 (broadcast along m) on eviction. Ditto h1T.
                h0Tc = hpool.tile([P, KC, BC], F32R, tag="h0Tc")
                for m in range(KC):
                    ps = psum.tile([P, BC], F32, tag="ps_down")
                    for kc in range(KC):
                        nc.tensor.matmul(
                            ps[:],
                            down0_sb[:, kc, m * P : (m + 1) * P],
                            hTc[:, kc],
                            start=(kc == 0),
                            stop=(kc == KC - 1),
                        )
                    nc.vector.tensor_mul(out=h0Tc[:, m], in0=ps[:], in1=m0c[:])

                h1Tc = hpool.tile([P, KC1, BC], F32R, tag="h1Tc")
                for m in range(KC1):
                    ps = psum.tile([P, BC], F32, tag="ps_down")
                    for kc in range(KC):
                        nc.tensor.matmul(
                            ps[:],
                            down1_sb[:, kc, m * P : (m + 1) * P],
                            hTc[:, kc],
                            start=(kc == 0),
                            stop=(kc == KC - 1),
                        )
                    nc.vector.tensor_mul(out=h1Tc[:, m], in0=ps[:], in1=m1c[:])

                # Decode. Each (bt, vtile): psum[b 128, v] accumulated over k.
                for bt in range(BT):
                    btsl = slice(bt * P, (bt + 1) * P)
                    row0 = bc * BC + bt * P

                    def group(lhsT_sb, nk, rhs_sb, vsl, ncols, col0):
                        ps = psum.tile([P, VT], F32, tag="ps_dec")
                        for kc in range(nk):
                            nc.tensor.matmul(
                                ps[:, :ncols],
                                lhsT_sb[:, kc, btsl],
                                rhs_sb[:, kc, vsl],
                                start=(kc == 0),
                                stop=(kc == nk - 1),
                            )
                        ot = opool.tile([P, VT], F32, tag="ot")
                        nc.vector.tensor_copy(out=ot[:, :ncols], in_=ps[:, :ncols])
                        nc.sync.dma_start(
                            out[row0 : row0 + P, col0 : col0 + ncols],
                            ot[:, :ncols],
                        )

                    group(hTc, KC, wh_sb, slice(0, HEAD_C), HEAD_C, 0)
                    for vt in range(T0_VT):
                        group(
                            h0Tc,
                            KC,
                            d0_sb,
                            slice(vt * VT, (vt + 1) * VT),
                            VT,
                            HEAD_C + vt * VT,
                        )
                    for vt in range(T1_VT):
                        group(
                            h1Tc,
                            KC1,
                            d1_sb,
                            slice(vt * VT, (vt + 1) * VT),
                            VT,
                            HEAD_C + T0_C + vt * VT,
                        )

    nc.compile()
    return nc


def _get_compiled():
    global _compiled
    if _compiled is None:
        _compiled = _build()
    return _compiled


def _prep_inputs(hidden, target, head_w, down0, dec0, down1, dec1):
    f32 = np.float32
    hidden = np.asarray(hidden, dtype=f32)
    target = np.asarray(target)
    head_w = np.asarray(head_w, dtype=f32)
    down0 = np.asarray(down0, dtype=f32)
    dec0 = np.asarray(dec0, dtype=f32)
    down1 = np.asarray(down1, dtype=f32)
    dec1 = np.asarray(dec1, dtype=f32)

    hT = np.ascontiguousarray(hidden.T)
    whp = np.zeros((H, HEAD_PAD), dtype=f32)
    whp[:, :HEAD] = head_w
    # Reference masks: cutoffs [2000, 10000, 50000] on raw target values.
    m0 = ((target >= 2000) & (target < 10000)).astype(f32)
    m1 = ((target >= 10000) & (target < 50000)).astype(f32)
    m0b = np.ascontiguousarray(np.broadcast_to(m0[None, :], (P, B)))
    m1b = np.ascontiguousarray(np.broadcast_to(m1[None, :], (P, B)))

    in_maps = []
    for c in range(NCORES):
        in_maps.append(
            {
                "hT": hT,
                "wh": np.ascontiguousarray(whp[:, c * HEAD_C : (c + 1) * HEAD_C]),
                "down0": down0,
                "down1": down1,
                "d0": np.ascontiguousarray(dec0[:, c * T0_C : (c + 1) * T0_C]),
                "d1": np.ascontiguousarray(dec1[:, c * T1_C : (c + 1) * T1_C]),
                "m0b": m0b,
                "m1b": m1b,
            }
        )
    return in_maps


def _assemble(results):
    full = np.empty((B, HEAD + T0 + T1), dtype=np.float32)
    for c in range(NCORES):
        o = results[c]["out"]
        hc = o[:, :HEAD_C]
        lo, hi = c * HEAD_C, (c + 1) * HEAD_C
        if lo < HEAD:
            full[:, lo : min(hi, HEAD)] = hc[:, : min(hi, HEAD) - lo]
        full[:, HEAD + c * T0_C : HEAD + (c + 1) * T0_C] = o[
            :, HEAD_C : HEAD_C + T0_C
        ]
        full[:, HEAD + T0 + c * T1_C : HEAD + T0 + (c + 1) * T1_C] = o[
            :, HEAD_C + T0_C :
        ]
    return full


def run_on_device(inputs, trace=False, trace_cores=None):
    """Run the SPMD kernel; returns (full_output, BassKernelResults)."""
    nc = _get_compiled()
    in_maps = _prep_inputs(**inputs)
    res = run_bass_kernel_spmd(
        nc,
        in_maps,
        list(range(NCORES)),
        trace=trace,
        trace_cores=trace_cores,
    )
    return _assemble(res.results), res


def kernel(**inputs) -> np.ndarray:
    full, _ = run_on_device(inputs)
    return full



# revision 4
# speedup vs baseline: 2.0371x; 2.0371x over previous
"""Adaptive-softmax logits kernel for trn2 (8 NeuronCores, SPMD).

Problem: out = concat([hidden @ head_w,
                       ((hidden @ down0) @ dec0) * m0,
                       ((hidden @ down1) @ dec1) * m1], axis=1)
with hidden [2048, 1024], head_w [1024, 2002], dec0 [1024, 8000],
dec1 [256, 40000]; m0/m1 are per-row cluster masks from `target`.

Strategy (v2):
- Exploit adaptive-softmax sparsity: only rows with target in
  [2000,10000) need tail-0 logits (~322 of 2048) and rows in
  [10000,50000) need tail-1 (~1645). The host permutes the batch so
  cluster-1 rows come first and cluster-0 rows last; the device then
  computes tail-1 on an aligned row-window [0, 1792) and tail-0 on
  [1536, 2048) only. No masks anywhere: the host scatters just the
  real cluster rows into a zero-initialized output.
- Vocab-parallel across the 8 cores: each core takes 1/8 of the head
  (padded 2002->2048), dec0 and dec1 columns, and computes the (tiny)
  down-projections redundantly for its window.
- All HBM I/O in bf16 (PE is 1 cycle/row for bf16 = same as fp32r,
  but DMA bytes halve; abs-max rel err stays ~2e-3, well inside the
  2e-2 gate). PSUM accumulates in fp32. Outputs upcast on the host.
- Host pre-swizzles every input to the exact SBUF layout
  [128, kchunks, free] so each DMA is 128 long contiguous runs.
- PSUM->SBUF evictions are split between the Vector and Scalar
  engines so neither becomes the bottleneck.

If the (deterministic) cluster counts ever exceed the compiled
windows, a numpy fallback keeps the result correct.
"""

import numpy as np
import ml_dtypes

import concourse.mybir as mybir
import concourse.tile as tile
from concourse import bacc
from concourse.bass_utils import run_bass_kernel_spmd

BF16 = ml_dtypes.bfloat16

# Problem shapes (hardcoded per the grading contract).
B = 2048  # batch
H = 1024  # hidden
NCORES = 8
P = 128
KC = H // P  # 8 k-chunks for K=1024 contractions
HEAD = 2002
HEAD_PAD = 2048
T0 = 8000  # cluster-0 decode width
T1 = 40000  # cluster-1 decode width
R1 = 256  # tail-1 down-projection width
KC1 = R1 // P  # 2 k-chunks for the t1 decode contraction
CUT0, CUT1, CUT2 = 2000, 10000, 50000

# Per-core column counts.
HEAD_C = HEAD_PAD // NCORES  # 256
T0_C = T0 // NCORES  # 1000
T1_C = T1 // NCORES  # 5000

# Row windows in permuted batch space (multiples of 128).
T1ROWS = 1792  # covers cluster-1 rows [0, c1); c1 ~ 1645
T0ROWS = 512  # window [B-512, B) covers cluster-0 rows; c0 ~ 322
T1BT = T1ROWS // P  # 14
T0BT = T0ROWS // P  # 4
BT = B // P  # 16

VT = 500  # decode free-dim tile
T0_VT = T0_C // VT  # 2
T1_VT = T1_C // VT  # 10

F32 = mybir.dt.float32
BF = mybir.dt.bfloat16

_compiled = None



def _evict(nc, use_scalar, out, in_):
    if use_scalar:
        nc.scalar.activation(out, in_, mybir.ActivationFunctionType.Copy)
    else:
        nc.vector.tensor_copy(out=out, in_=in_)

def _build():
    nc = bacc.Bacc(None)

    # All inputs host-pre-swizzled to [128, kchunk, free] (bf16).
    hT = nc.declare_dram_parameter("hT", [P, KC, B], BF, isOutput=False)
    wh = nc.declare_dram_parameter("wh", [P, KC, HEAD_C], BF, isOutput=False)
    down0 = nc.declare_dram_parameter("down0", [P, KC, H], BF, isOutput=False)
    down1 = nc.declare_dram_parameter("down1", [P, KC, R1], BF, isOutput=False)
    d0 = nc.declare_dram_parameter("d0", [P, KC, T0_C], BF, isOutput=False)
    d1 = nc.declare_dram_parameter("d1", [P, KC1, T1_C], BF, isOutput=False)
    # Outputs (bf16). oh/o0 mirror their SBUF staging layout; host fixes.
    o1 = nc.declare_dram_parameter("o1", [T1ROWS, T1_C], BF, isOutput=True)
    oh = nc.declare_dram_parameter("oh", [P, BT, HEAD_C], BF, isOutput=True)
    o0 = nc.declare_dram_parameter("o0", [P, T0BT, T0_C], BF, isOutput=True)

    with tile.TileContext(nc) as tc:
        with (
            tc.tile_pool(name="consts", bufs=1) as consts,
            tc.tile_pool(name="acts", bufs=1) as acts,
            tc.tile_pool(name="o1stage", bufs=2) as o1stage,
            tc.tile_pool(name="psh", bufs=2, space="PSUM") as psh,
            tc.tile_pool(name="psd", bufs=6, space="PSUM") as psd,
        ):
            # Resident inputs, in the order compute needs them.
            down1_sb = consts.tile([P, KC, R1], BF)
            nc.sync.dma_start(down1_sb[:], down1[:])
            hT_sb = consts.tile([P, KC, B], BF)
            nc.sync.dma_start(hT_sb[:], hT[:])
            d1_sb = consts.tile([P, KC1, T1_C], BF)
            nc.sync.dma_start(d1_sb[:], d1[:])
            down0_sb = consts.tile([P, KC, H], BF)
            nc.sync.dma_start(down0_sb[:], down0[:])
            d0_sb = consts.tile([P, KC, T0_C], BF)
            nc.sync.dma_start(d0_sb[:], d0[:])
            wh_sb = consts.tile([P, KC, HEAD_C], BF)
            nc.sync.dma_start(wh_sb[:], wh[:])

            # h1T[f, kc1, b] = sum_k down1[k, f] hT[k, b], b in [0, T1ROWS).
            h1T = acts.tile([P, KC1, T1ROWS], BF)
            for mc in range(KC1):
                for b0 in range(0, T1ROWS, 512):
                    bn = min(512, T1ROWS - b0)
                    ps = psh.tile([P, 512], F32, tag="psh")
                    for kc in range(KC):
                        nc.tensor.matmul(
                            ps[:, :bn],
                            down1_sb[:, kc, mc * P : (mc + 1) * P],
                            hT_sb[:, kc, b0 : b0 + bn],
                            start=(kc == 0),
                            stop=(kc == KC - 1),
                        )
                    nc.vector.tensor_copy(
                        out=h1T[:, mc, b0 : b0 + bn], in_=ps[:, :bn]
                    )

            # Tail-1 decode: out[b, v] over window rows, 1/8 vocab cols.
            for bt in range(T1BT):
                btsl = slice(bt * P, (bt + 1) * P)
                stage = o1stage.tile([P, T1_C], BF, tag="o1s")
                for vt in range(T1_VT):
                    vsl = slice(vt * VT, (vt + 1) * VT)
                    ps = psd.tile([P, VT], F32, tag="psd")
                    for kc in range(KC1):
                        nc.tensor.matmul(
                            ps[:],
                            h1T[:, kc, btsl],
                            d1_sb[:, kc, vsl],
                            start=(kc == 0),
                            stop=(kc == KC1 - 1),
                        )
                    _evict(nc, vt % 3 == 2, stage[:, vsl], ps[:])
                nc.sync.dma_start(o1[btsl, :], stage[:])

            # h0T[f, kc, b] for the tail-0 window rows [B-T0ROWS, B).
            h0T = acts.tile([P, KC, T0ROWS], BF)
            for mc in range(KC):
                ps = psh.tile([P, 512], F32, tag="psh")
                for kc in range(KC):
                    nc.tensor.matmul(
                        ps[:, :T0ROWS],
                        down0_sb[:, kc, mc * P : (mc + 1) * P],
                        hT_sb[:, kc, B - T0ROWS : B],
                        start=(kc == 0),
                        stop=(kc == KC - 1),
                    )
                nc.vector.tensor_copy(out=h0T[:, mc, :], in_=ps[:, :T0ROWS])

            # Head: all B rows, 1/8 of (padded) head cols.
            stageh = acts.tile([P, BT, HEAD_C], BF)
            for bt in range(BT):
                btsl = slice(bt * P, (bt + 1) * P)
                ps = psd.tile([P, VT], F32, tag="psd")
                for kc in range(KC):
                    nc.tensor.matmul(
                        ps[:, :HEAD_C],
                        hT_sb[:, kc, btsl],
                        wh_sb[:, kc, :],
                        start=(kc == 0),
                        stop=(kc == KC - 1),
                    )
                _evict(nc, bt % 2 == 0, stageh[:, bt, :], ps[:, :HEAD_C])
            nc.sync.dma_start(oh[:], stageh[:])

            # Tail-0 decode over its window.
            stage0 = acts.tile([P, T0BT, T0_C], BF)
            for bt in range(T0BT):
                btsl = slice(bt * P, (bt + 1) * P)
                for vt in range(T0_VT):
                    vsl = slice(vt * VT, (vt + 1) * VT)
                    ps = psd.tile([P, VT], F32, tag="psd")
                    for kc in range(KC):
                        nc.tensor.matmul(
                            ps[:],
                            h0T[:, kc, btsl],
                            d0_sb[:, kc, vsl],
                            start=(kc == 0),
                            stop=(kc == KC - 1),
                        )
                    _evict(nc, vt % 2 == 0, stage0[:, bt, vsl], ps[:])
            nc.sync.dma_start(o0[:], stage0[:])

    nc.compile()
    return nc


def _get_compiled():
    global _compiled
    if _compiled is None:
        _compiled = _build()
    return _compiled


def _swz(a, kchunks):
    """[K, N] row-major -> [128, kchunks, N] (bf16, contiguous)."""
    k, n = a.shape
    assert k == kchunks * P
    return np.ascontiguousarray(
        a.reshape(kchunks, P, n).transpose(1, 0, 2).astype(BF16)
    )


def _numpy_fallback(hidden, target, head_w, down0, dec0, down1, dec1):
    head = hidden @ head_w
    m0 = ((target >= CUT0) & (target < CUT1)).astype(hidden.dtype)
    m1 = ((target >= CUT1) & (target < CUT2)).astype(hidden.dtype)
    t0 = ((hidden @ down0) @ dec0) * m0[:, None]
    t1 = ((hidden @ down1) @ dec1) * m1[:, None]
    return np.concatenate([head, t0, t1], axis=1).astype(np.float32)


def _prep(hidden, target, head_w, down0, dec0, down1, dec1):
    f32 = np.float32
    hidden = np.asarray(hidden, dtype=f32)
    target = np.asarray(target)
    head_w = np.asarray(head_w, dtype=f32)
    down0 = np.asarray(down0, dtype=f32)
    dec0 = np.asarray(dec0, dtype=f32)
    down1 = np.asarray(down1, dtype=f32)
    dec1 = np.asarray(dec1, dtype=f32)

    in1 = (target >= CUT1) & (target < CUT2)
    in0 = (target >= CUT0) & (target < CUT1)
    idx1 = np.nonzero(in1)[0]
    idx0 = np.nonzero(in0)[0]
    idxr = np.nonzero(~(in0 | in1))[0]
    c1, c0 = len(idx1), len(idx0)
    if c1 > T1ROWS or c0 > T0ROWS:
        return None, None  # pathological inputs: numpy fallback

    # Permuted batch: [cluster-1 | rest | cluster-0].
    perm = np.concatenate([idx1, idxr, idx0])
    hp = hidden[perm]  # [B, H]
    hT = _swz(np.ascontiguousarray(hp.T), KC)  # [128, 8, B]

    whp = np.zeros((H, HEAD_PAD), dtype=f32)
    whp[:, :HEAD] = head_w
    down0_s = _swz(down0, KC)
    down1_s = _swz(down1, KC)

    in_maps = []
    for c in range(NCORES):
        in_maps.append(
            {
                "hT": hT,
                "wh": _swz(whp[:, c * HEAD_C : (c + 1) * HEAD_C], KC),
                "down0": down0_s,
                "down1": down1_s,
                "d0": _swz(dec0[:, c * T0_C : (c + 1) * T0_C], KC),
                "d1": _swz(dec1[:, c * T1_C : (c + 1) * T1_C], KC1),
            }
        )
    return in_maps, (perm, c1, c0)


def _assemble(results, meta):
    perm, c1, c0 = meta
    f32 = np.float32
    outp = np.zeros((B, HEAD + T0 + T1), dtype=f32)
    for c in range(NCORES):
        r = results[c]
        # head: oh [128, 16, 256] -> rows bt*128+p
        head_c = np.asarray(r["oh"]).transpose(1, 0, 2).reshape(B, HEAD_C)
        lo = c * HEAD_C
        hi = min(lo + HEAD_C, HEAD)
        if lo < HEAD:
            outp[:, lo:hi] = head_c[:, : hi - lo].astype(f32)
        if c0:
            t0_c = np.asarray(r["o0"]).transpose(1, 0, 2).reshape(T0ROWS, T0_C)
            outp[B - c0 :, HEAD + c * T0_C : HEAD + (c + 1) * T0_C] = t0_c[
                T0ROWS - c0 :
            ].astype(f32)
        if c1:
            outp[:c1, HEAD + T0 + c * T1_C : HEAD + T0 + (c + 1) * T1_C] = (
                np.asarray(r["o1"])[:c1].astype(f32)
            )
    out = np.empty_like(outp)
    out[perm] = outp
    return out


def run_on_device(inputs, trace=False, trace_cores=None):
    """Run the SPMD kernel; returns (full_output, BassKernelResults)."""
    in_maps, meta = _prep(**inputs)
    if in_maps is None:
        return _numpy_fallback(**{k: np.asarray(v) for k, v in inputs.items()}), None
    nc = _get_compiled()
    res = run_bass_kernel_spmd(
        nc,
        in_maps,
        list(range(NCORES)),
        trace=trace,
        trace_cores=trace_cores,
    )
    return _assemble(res.results, meta), res


def kernel(**inputs) -> np.ndarray:
    full, _ = run_on_device(inputs)
    return full


# revision 5
# speedup vs baseline: 2.3336x; 1.1455x over previous
"""Adaptive-softmax logits kernel for trn2 (8 NeuronCores, SPMD).

Problem: out = concat([hidden @ head_w,
                       ((hidden @ down0) @ dec0) * m0,
                       ((hidden @ down1) @ dec1) * m1], axis=1)
with hidden [2048, 1024], head_w [1024, 2002], dec0 [1024, 8000],
dec1 [256, 40000]; m0/m1 are per-row cluster masks from `target`.

Strategy:
- Exploit adaptive-softmax sparsity: only rows with target in
  [2000,10000) need tail-0 logits (~322 of 2048) and rows in
  [10000,50000) need tail-1 (~1645). The host permutes the batch so
  cluster-1 rows come first and cluster-0 rows last; the device
  computes tail-1 only on rows [0, ceil128(c1)) and tail-0 only on
  rows [B - ceil128(c0), B). No masks anywhere: the host scatters
  just the real cluster rows into a zero-initialized output. The
  kernel is compiled for the (deterministic) rounded row counts and
  cached per shape.
- Vocab-parallel across the 8 cores: each core takes 1/8 of the head
  (padded 2002->2048), dec0 and dec1 columns, and computes the small
  down-projections redundantly for its window.
- All HBM I/O in bf16 (PE is 1 cycle/row for bf16 = same as fp32r,
  but DMA bytes halve; abs-max rel err ~4e-3, well inside the 2e-2
  gate). PSUM accumulates in fp32. Outputs upcast on the host.
- Host pre-swizzles every input to the exact SBUF layout
  [128, kchunks, free]; hidden is split into 4 column chunks so
  compute starts as soon as the first chunk lands.
- PSUM->SBUF evictions use 2-bank psum tiles (two matmuls, one
  copy) and are split between the Vector and Scalar engines.

If the cluster counts exceed the window caps, a numpy fallback
keeps the result correct.
"""

import numpy as np
import ml_dtypes

import concourse.mybir as mybir
import concourse.tile as tile
from concourse import bacc
from concourse.bass_utils import run_bass_kernel_spmd

BF16 = ml_dtypes.bfloat16

# Problem shapes (hardcoded per the grading contract).
B = 2048  # batch
H = 1024  # hidden
NCORES = 8
P = 128
KC = H // P  # 8 k-chunks for K=1024 contractions
KC1 = 2  # k-chunks for the t1 decode contraction (K=256)
HEAD = 2002
HEAD_PAD = 2048
T0 = 8000  # cluster-0 decode width
T1 = 40000  # cluster-1 decode width
R1 = 256  # tail-1 down-projection width
CUT0, CUT1, CUT2 = 2000, 10000, 50000

HEAD_C = HEAD_PAD // NCORES  # 256
T0_C = T0 // NCORES  # 1000
T1_C = T1 // NCORES  # 5000

BT = B // P  # 16
HCH = 4  # hT column chunks
HCW = B // HCH  # 512

VT = 500  # decode free-dim tile (within one psum bank)
PAIR = 1024  # 2-bank psum tile width (fp32)

F32 = mybir.dt.float32
BF = mybir.dt.bfloat16
COPY = mybir.ActivationFunctionType.Copy

_compiled = {}


def _evict(nc, use_scalar, out, in_):
    if use_scalar:
        nc.scalar.activation(out, in_, COPY)
    else:
        nc.vector.tensor_copy(out=out, in_=in_)


def _build(t1rows, t0rows):
    t1bt = t1rows // P
    t0bt = t0rows // P

    nc = bacc.Bacc(None)

    # Inputs host-pre-swizzled to [128, kchunk, free] (bf16); hT split
    # into HCH column chunks, DMA'd in compute order.
    hT = nc.declare_dram_parameter("hT", [HCH, P, KC, HCW], BF, isOutput=False)
    wh = nc.declare_dram_parameter("wh", [P, KC, HEAD_C], BF, isOutput=False)
    down0 = nc.declare_dram_parameter("down0", [P, KC, H], BF, isOutput=False)
    down1 = nc.declare_dram_parameter("down1", [P, KC, R1], BF, isOutput=False)
    d0 = nc.declare_dram_parameter("d0", [P, KC, T0_C], BF, isOutput=False)
    d1 = nc.declare_dram_parameter("d1", [P, KC1, T1_C], BF, isOutput=False)
    # Outputs (bf16). oh/o0 mirror their SBUF staging layout; host fixes.
    o1 = nc.declare_dram_parameter("o1", [t1rows, T1_C], BF, isOutput=True)
    oh = nc.declare_dram_parameter("oh", [P, BT, HEAD_C], BF, isOutput=True)
    o0 = nc.declare_dram_parameter("o0", [P, t0bt, T0_C], BF, isOutput=True)

    with tile.TileContext(nc) as tc:
        with (
            tc.tile_pool(name="consts", bufs=1) as consts,
            tc.tile_pool(name="acts", bufs=1) as acts,
            tc.tile_pool(name="o1stage", bufs=2) as o1stage,
            tc.tile_pool(name="psh", bufs=2, space="PSUM") as psh,
            tc.tile_pool(name="psd", bufs=3, space="PSUM") as psd,
        ):
            # Resident inputs, in the order compute needs them.
            down1_sb = consts.tile([P, KC, R1], BF)
            nc.sync.dma_start(down1_sb[:], down1[:])
            hT_sb = []
            for i in range(HCH):
                t = consts.tile([P, KC, HCW], BF, tag=f"hT{i}")
                nc.sync.dma_start(t[:], hT[i])
                hT_sb.append(t)
            wh_sb = consts.tile([P, KC, HEAD_C], BF)
            nc.sync.dma_start(wh_sb[:], wh[:])
            d1_sb = consts.tile([P, KC1, T1_C], BF)
            nc.sync.dma_start(d1_sb[:], d1[:])
            down0_sb = consts.tile([P, KC, H], BF)
            nc.sync.dma_start(down0_sb[:], down0[:])
            d0_sb = consts.tile([P, KC, T0_C], BF)
            nc.sync.dma_start(d0_sb[:], d0[:])

            def hslice(b0, bn):
                """(chunk tile, column slice) for permuted cols [b0, b0+bn)."""
                c = b0 // HCW
                off = b0 - c * HCW
                assert off + bn <= HCW
                return hT_sb[c], slice(off, off + bn)

            # h1T[f, kc1, b] = sum_k down1[k, f] hT[k, b], b in [0, t1rows).
            h1T = acts.tile([P, KC1, t1rows], BF)
            for b0 in range(0, t1rows, HCW):
                bn = min(HCW, t1rows - b0)
                ht, hsl = hslice(b0, bn)
                for mc in range(KC1):
                    ps = psh.tile([P, 512], F32, tag="psh")
                    for kc in range(KC):
                        nc.tensor.matmul(
                            ps[:, :bn],
                            down1_sb[:, kc, mc * P : (mc + 1) * P],
                            ht[:, kc, hsl],
                            start=(kc == 0),
                            stop=(kc == KC - 1),
                        )
                    nc.vector.tensor_copy(
                        out=h1T[:, mc, b0 : b0 + bn], in_=ps[:, :bn]
                    )

            # Head: all B rows, 1/8 of (padded) head cols.
            stageh = acts.tile([P, BT, HEAD_C], BF)
            for bt in range(BT):
                ht, hsl = hslice(bt * P, P)
                ps = psh.tile([P, 512], F32, tag="psh")
                for kc in range(KC):
                    nc.tensor.matmul(
                        ps[:, :HEAD_C],
                        ht[:, kc, hsl],
                        wh_sb[:, kc, :],
                        start=(kc == 0),
                        stop=(kc == KC - 1),
                    )
                _evict(nc, bt % 2 == 0, stageh[:, bt, :], ps[:, :HEAD_C])
            nc.sync.dma_start(oh[:], stageh[:])

            # Tail-1 decode: out[b, v] over window rows, 1/8 vocab cols.
            # 2-bank psum tiles: two N=500 matmul groups per tile, one
            # strided eviction per pair.
            for bt in range(t1bt):
                btsl = slice(bt * P, (bt + 1) * P)
                stage = o1stage.tile([P, T1_C], BF, tag="o1s")
                for vp in range(T1_C // (2 * VT)):  # 5 pairs
                    ps = psd.tile([P, PAIR], F32, tag="psd")
                    for half in range(2):
                        vt = vp * 2 + half
                        vsl = slice(vt * VT, (vt + 1) * VT)
                        psl = slice(half * 512, half * 512 + VT)
                        for kc in range(KC1):
                            nc.tensor.matmul(
                                ps[:, psl],
                                h1T[:, kc, btsl],
                                d1_sb[:, kc, vsl],
                                start=(kc == 0),
                                stop=(kc == KC1 - 1),
                            )
                    pv = ps[:].rearrange("p (two v) -> p two v", two=2)
                    _evict(
                        nc,
                        vp >= 3,
                        stage[:, vp * 2 * VT : (vp + 1) * 2 * VT],
                        pv[:, :, :VT],
                    )
                nc.sync.dma_start(o1[btsl, :], stage[:])

            # h0T[f, kc, b] for the tail-0 window rows [B-t0rows, B).
            h0T = acts.tile([P, KC, t0rows], BF)
            ht0, hsl0 = hslice(B - t0rows, t0rows)
            for mc in range(KC):
                ps = psh.tile([P, 512], F32, tag="psh")
                for kc in range(KC):
                    nc.tensor.matmul(
                        ps[:, :t0rows],
                        down0_sb[:, kc, mc * P : (mc + 1) * P],
                        ht0[:, kc, hsl0],
                        start=(kc == 0),
                        stop=(kc == KC - 1),
                    )
                nc.vector.tensor_copy(out=h0T[:, mc, :], in_=ps[:, :t0rows])

            # Tail-0 decode over its window.
            stage0 = acts.tile([P, t0bt, T0_C], BF)
            for bt in range(t0bt):
                btsl = slice(bt * P, (bt + 1) * P)
                ps = psd.tile([P, PAIR], F32, tag="psd")
                for half in range(2):
                    vsl = slice(half * VT, (half + 1) * VT)
                    psl = slice(half * 512, half * 512 + VT)
                    for kc in range(KC):
                        nc.tensor.matmul(
                            ps[:, psl],
                            h0T[:, kc, btsl],
                            d0_sb[:, kc, vsl],
                            start=(kc == 0),
                            stop=(kc == KC - 1),
                        )
                pv = ps[:].rearrange("p (two v) -> p two v", two=2)
                _evict(nc, bt % 2 == 0, stage0[:, bt, :], pv[:, :, :VT])
            nc.sync.dma_start(o0[:], stage0[:])

    nc.compile()
    return nc


def _get_compiled(t1rows, t0rows):
    key = (t1rows, t0rows)
    if key not in _compiled:
        _compiled[key] = _build(*key)
    return _compiled[key]


def _swz(a, kchunks):
    """[K, N] row-major -> [128, kchunks, N] (bf16, contiguous)."""
    k, n = a.shape
    assert k == kchunks * P
    return np.ascontiguousarray(
        a.reshape(kchunks, P, n).transpose(1, 0, 2).astype(BF16)
    )


def _numpy_fallback(hidden, target, head_w, down0, dec0, down1, dec1):
    head = hidden @ head_w
    m0 = ((target >= CUT0) & (target < CUT1)).astype(hidden.dtype)
    m1 = ((target >= CUT1) & (target < CUT2)).astype(hidden.dtype)
    t0 = ((hidden @ down0) @ dec0) * m0[:, None]
    t1 = ((hidden @ down1) @ dec1) * m1[:, None]
    return np.concatenate([head, t0, t1], axis=1).astype(np.float32)


def _ceil128(n):
    return max(P, -(-n // P) * P)


def _prep(hidden, target, head_w, down0, dec0, down1, dec1):
    f32 = np.float32
    hidden = np.asarray(hidden, dtype=f32)
    target = np.asarray(target)
    head_w = np.asarray(head_w, dtype=f32)
    down0 = np.asarray(down0, dtype=f32)
    dec0 = np.asarray(dec0, dtype=f32)
    down1 = np.asarray(down1, dtype=f32)
    dec1 = np.asarray(dec1, dtype=f32)

    in1 = (target >= CUT1) & (target < CUT2)
    in0 = (target >= CUT0) & (target < CUT1)
    idx1 = np.nonzero(in1)[0]
    idx0 = np.nonzero(in0)[0]
    idxr = np.nonzero(~(in0 | in1))[0]
    c1, c0 = len(idx1), len(idx0)
    t1rows, t0rows = _ceil128(c1), _ceil128(c0)
    if t0rows > HCW or t1rows > B - t0rows:
        return None, None  # windows collide: numpy fallback

    # Permuted batch: [cluster-1 | rest | cluster-0].
    perm = np.concatenate([idx1, idxr, idx0])
    hp = hidden[perm]  # [B, H]
    hTs = _swz(np.ascontiguousarray(hp.T), KC)  # [128, 8, B]
    hTc = np.ascontiguousarray(
        hTs.reshape(P, KC, HCH, HCW).transpose(2, 0, 1, 3)
    )  # [HCH, 128, 8, HCW]

    whp = np.zeros((H, HEAD_PAD), dtype=f32)
    whp[:, :HEAD] = head_w
    down0_s = _swz(down0, KC)
    down1_s = _swz(down1, KC)

    in_maps = []
    for c in range(NCORES):
        in_maps.append(
            {
                "hT": hTc,
                "wh": _swz(whp[:, c * HEAD_C : (c + 1) * HEAD_C], KC),
                "down0": down0_s,
                "down1": down1_s,
                "d0": _swz(dec0[:, c * T0_C : (c + 1) * T0_C], KC),
                "d1": _swz(dec1[:, c * T1_C : (c + 1) * T1_C], KC1),
            }
        )
    return in_maps, (perm, c1, c0, t1rows, t0rows)


def _assemble(results, meta):
    perm, c1, c0, t1rows, t0rows = meta
    f32 = np.float32
    outp = np.zeros((B, HEAD + T0 + T1), dtype=f32)
    for c in range(NCORES):
        r = results[c]
        head_c = np.asarray(r["oh"]).transpose(1, 0, 2).reshape(B, HEAD_C)
        lo = c * HEAD_C
        hi = min(lo + HEAD_C, HEAD)
        if lo < HEAD:
            outp[:, lo:hi] = head_c[:, : hi - lo].astype(f32)
        if c0:
            t0_c = np.asarray(r["o0"]).transpose(1, 0, 2).reshape(t0rows, T0_C)
            outp[B - c0 :, HEAD + c * T0_C : HEAD + (c + 1) * T0_C] = t0_c[
                t0rows - c0 :
            ].astype(f32)
        if c1:
            outp[:c1, HEAD + T0 + c * T1_C : HEAD + T0 + (c + 1) * T1_C] = (
                np.asarray(r["o1"])[:c1].astype(f32)
            )
    out = np.empty_like(outp)
    out[perm] = outp
    return out


def run_on_device(inputs, trace=False, trace_cores=None):
    """Run the SPMD kernel; returns (full_output, BassKernelResults)."""
    in_maps, meta = _prep(**inputs)
    if in_maps is None:
        return _numpy_fallback(**{k: np.asarray(v) for k, v in inputs.items()}), None
    nc = _get_compiled(meta[3], meta[4])
    res = run_bass_kernel_spmd(
        nc,
        in_maps,
        list(range(NCORES)),
        trace=trace,
        trace_cores=trace_cores,
    )
    return _assemble(res.results, meta), res


def kernel(**inputs) -> np.ndarray:
    full, _ = run_on_device(inputs)
    return full
